# revision 1
# baseline (speedup 1.0000x reference)
"""CP-factorized embedding lookup on 8 TRN2 NeuronCores.

Reference computes full[a,b,c,d,e,f] = sum_r U0[a,r]*...*U5[f,r], reshapes to a
(50000, 512) table, and gathers rows by x. We never materialize the table:

  out[n, e] = sum_r (U0[a_n,r]*U1[b_n,r]*U2[c_n,r]) * (U3[d,r]*U4[e2,r]*U5[f,r])
            = sum_r V[n, r] * W[e, r]

with v = 1000a + 25b + c and e = 64d + 8e2 + f.

Per core (1024 indices, data-parallel over the 8192 total), in two pipelined
512-index halves:
  1. broadcast x across 115 partitions (50+40+25 stacked factor rows) and
     decompose it in place with per-partition constants in a short 16-bit
     DVE chain (4x perf mode):
       rows  0:50  -> a      = floor(v/1000)   (1000 when v == 0: see below)
       rows 50:90  -> b + 50 = floor(v/25) - 40*floor(v/1000) + 50
       rows 90:115 -> c + 90 = (v-25000) - 25*(floor(v/25)-1000) + 90
     floor(v/d) = f32->i16 cast of (v + bias)*(1/d); the HW cast rounds to
     nearest even, bias = -(d/2 - 0.5) puts the value mid-interval, so the
     result is exact. Block 2 is offset by -25000 to fit int16. The
     padding mask is folded in: rows 0:50 use s2 = min(v, 1) and
     diff = a - 1000*s2 + 1000, which equals a for v > 0 and 1000 (no
     one-hot hit -> zero row) for v == 0.
  2. one-hot[115, 512] = is_equal(diff, iota); gather via one PE matmul
     with block-diag stacked [U0;U1;U2] as lhsT -> psum[96, 512];
     V = elementwise product of the three 32-row blocks
  3. W[32, 512] = Khatri-Rao of U3,U4,U5 built with two broadcast multiplies
     (U3/U4/U5 transposed on-chip through the PE)
  4. out chunk c: matmul(lhsT=V[:,128j:128j+128], rhs=W) -> psum, two chunks
     batched per [128, 1024] psum pair, one Scalar-engine copy -> SBUF,
     one DMA per 256 output rows

All small constant operands (decomposition table, iota, identity, stacked
U3..U5, block-diagonal [U0;U1;U2]) are packed host-side into one aux input
(pure rearrangement/zero-padding -- all arithmetic stays on device) so the
front end costs a single small DMA. Matmul operands are produced as float32r
(tf32-like, 1 row/cycle vs 4 for float32); one-hot entries are exact in any
dtype and the factor rounding error is ~1e-4 relative, far inside tolerance.
"""

import numpy as np

import concourse.bass as bass
import concourse.mybir as mybir
import concourse.tile as tile
from concourse import bacc
from concourse.bass_utils import run_bass_kernel_spmd

F32 = mybir.dt.float32
F32R = mybir.dt.float32r
I32 = mybir.dt.int32
I16 = mybir.dt.int16
U16 = mybir.dt.uint16
ALU = mybir.AluOpType

N_CORES = 8
PER_CORE = 1024           # indices per core (8192 / 8)
HALF = 512                # pipeline granularity (one PSUM bank of columns)
EMB = 512
RANK = 32
KV = 115                  # 50 + 40 + 25 stacked vocab-factor rows
MV = 96                   # 3 * RANK stacked outputs

R1000 = float(np.float32(1.0 / 1000.0))
R25 = float(np.float32(1.0 / 25.0))

# aux layout: [115, 7 + 24 + 32 + 96]
CC_OFF = 0      # [115, 7] decomposition constants + iota
ID_OFF = 7      # [24, 24] identity (rows 0:24)
U345_OFF = 31   # [24, 32] stacked U3;U4;U5 (rows 0:24)
UBLK_OFF = 63   # [115, 96] block-diag [U0;U1;U2]
ONES_OFF = 159  # [1, 115] row of ones (lhsT of the broadcast matmul)
AUX_W = 274

# matmul operand dtype: float32r streams 1 row/cycle (vs 4 for float32).
MM_DT = F32R


def _const_table() -> np.ndarray:
    """[115, 7] per-partition constants: b1, R1, b2, R2, K, OFF, iota.

    Chain (s1, s2 are f32->i16 floor stages; the cast rounds to nearest):
      s1 = rint((v + b1) * R1);  s2 = rint((v + b2) * R2)
      (rows 0:50 overwrite: s2 = min(v, 1))
      diff = s1 - (K*s2 - OFF)  ; onehot = (diff == iota)
    """
    cc = np.zeros((KV, 7), np.float32)
    rows = ((0, 50), (50, 90), (90, 115))
    vals = [
        # s1 = a; s2 = min(v,1); hit iff a == 1000*s2 - 1000 + p
        (-499.5, R1000, 0.0, 1.0, 1000.0, 1000.0),
        # s1 = q25; s2 = a; hit iff q25 == 40a - 50 + p  (p abs. row 50..89)
        (-12.0, R25, -499.5, R1000, 40.0, 50.0),
        # s1 = v-25000; s2 = q25-1000; hit iff s1 == 25*s2 - 90 + p
        (-25000.0, 1.0, -25012.0, R25, 25.0, 90.0),
    ]
    for (lo, hi), v6 in zip(rows, vals):
        cc[lo:hi, 0:6] = np.float32(v6)
    # OFF2 = OFF - row: tkp = K*s2 - OFF2 and the one-hot becomes a single
    # fused tensor_tensor is_equal(s1, tkp)
    cc[:, 5] -= np.arange(KV, dtype=np.float32)
    return cc


def _aux_table(us: list[np.ndarray]) -> np.ndarray:
    aux = np.zeros((KV, AUX_W), np.float32)
    aux[:, CC_OFF:CC_OFF + 7] = _const_table()
    aux[0:24, ID_OFF:ID_OFF + 24] = np.eye(24, dtype=np.float32)
    aux[0:8, U345_OFF:U345_OFF + 32] = us[3]
    aux[8:16, U345_OFF:U345_OFF + 32] = us[4]
    aux[16:24, U345_OFF:U345_OFF + 32] = us[5]
    aux[0:50, UBLK_OFF:UBLK_OFF + 32] = us[0]
    aux[50:90, UBLK_OFF + 32:UBLK_OFF + 64] = us[1]
    aux[90:115, UBLK_OFF + 64:UBLK_OFF + 96] = us[2]
    aux[0, ONES_OFF:ONES_OFF + KV] = 1.0
    return aux


def build():
    nc = bacc.Bacc("TRN2", target_bir_lowering=False, debug=False)

    x = nc.dram_tensor("x", [PER_CORE], I32, kind="ExternalInput")
    aux_d = nc.dram_tensor("aux", [KV, AUX_W], F32, kind="ExternalInput")
    out = nc.dram_tensor("out", [PER_CORE, EMB], F32, kind="ExternalOutput")

    NH = PER_CORE // HALF   # 2 halves
    NC2 = HALF // 256       # 2 two-chunk groups per half

    with tile.TileContext(nc) as tc:
        with (
            tc.tile_pool(name="const", bufs=1) as cpool,
            tc.tile_pool(name="work", bufs=2) as wpool,
            tc.tile_pool(name="vpsum", bufs=2, space="PSUM") as ppool,
            tc.tile_pool(name="osb", bufs=2) as opool,
            tc.tile_pool(name="opsum", bufs=2, space="PSUM") as oppool,
        ):
            # ---- broadcast x across the 115 stacked factor rows (one
            # full-width DMA on the sync ring); aux lands in parallel on
            # the scalar ring.
            aux = cpool.tile([KV, AUX_W], F32)
            nc.sync.dma_start(out=aux[:], in_=aux_d[:])
            xrep = cpool.tile([KV, PER_CORE], I32)
            nc.sync.dma_start(
                out=xrep[:], in_=x[:].unsqueeze(0).partition_broadcast(KV)
            )
            cc = aux[:, CC_OFF:CC_OFF + 7]
            idm = aux[0:24, ID_OFF:ID_OFF + 24]
            u345 = aux[0:24, U345_OFF:U345_OFF + 32]

            # f32r-rounded copy of the block-diag factors for the gather mm
            ublk = cpool.tile([KV, MV], MM_DT)
            nc.vector.tensor_copy(out=ublk[:], in_=aux[:, UBLK_OFF:UBLK_OFF + 96])

            # ---- W[r, e] = U3[d,r] * U4[e2,r] * U5[f,r],  e = 64d + 8e2 + f
            u345t_ps = ppool.tile([RANK, 24], F32, tag="pv")
            nc.tensor.transpose(u345t_ps[:], u345, idm)
            u345t = cpool.tile([RANK, 24], F32)
            nc.scalar.copy(out=u345t[:], in_=u345t_ps[:])
            t45 = cpool.tile([RANK, 64], F32)
            nc.vector.tensor_tensor(
                out=t45[:].rearrange("r (e f) -> r e f", e=8),
                in0=u345t[:, 8:16].unsqueeze(2).broadcast_to([RANK, 8, 8]),
                in1=u345t[:, 16:24].unsqueeze(1).broadcast_to([RANK, 8, 8]),
                op=ALU.mult,
            )
            wt = cpool.tile([RANK, EMB], MM_DT)
            nc.vector.tensor_tensor(
                out=wt[:].rearrange("r (d ef) -> r d ef", d=8),
                in0=u345t[:, 0:8].unsqueeze(2).broadcast_to([RANK, 8, 64]),
                in1=t45[:].unsqueeze(1).broadcast_to([RANK, 8, 64]),
                op=ALU.mult,
            )

            # ---- full-width 5-op decomposition chain straight off the
            # int32 broadcast (mixed int-in/f32-scalar tensor_scalar is
            # exact on HW: internal fp32 ALU + round-to-nearest int cast)
            s1 = cpool.tile([KV, PER_CORE], I16)
            nc.vector.tensor_scalar(
                out=s1[:], in0=xrep[:], scalar1=cc[:, 0:1], scalar2=cc[:, 1:2],
                op0=ALU.add, op1=ALU.mult,
            )
            s2 = cpool.tile([KV, PER_CORE], I16)
            nc.vector.tensor_scalar(
                out=s2[:], in0=xrep[:], scalar1=cc[:, 2:3], scalar2=cc[:, 3:4],
                op0=ALU.add, op1=ALU.mult,
            )
            # rows 0:50: s2 = min(v, 1) -> folds the v==0 padding mask into
            # the block-0 one-hot (no hit for v == 0 -> zero output row)
            nc.vector.tensor_scalar(
                out=s2[0:50, :], in0=xrep[0:50, :], scalar1=1.0, scalar2=1.0,
                op0=ALU.min, op1=ALU.mult,
            )
            tkp = cpool.tile([KV, PER_CORE], I16)
            nc.vector.tensor_scalar(
                out=tkp[:], in0=s2[:], scalar1=cc[:, 4:5], scalar2=cc[:, 5:6],
                op0=ALU.mult, op1=ALU.subtract,
            )
            onehot = cpool.tile([KV, PER_CORE], MM_DT)
            nc.vector.tensor_tensor(
                out=onehot[:], in0=s1[:], in1=tkp[:], op=ALU.is_equal
            )

            for h in range(NH):
                pv = ppool.tile([MV, HALF], F32, name=f"pv_{h}", tag="pv")
                nc.tensor.matmul(
                    pv[:], lhsT=ublk[:],
                    rhs=onehot[:, h * HALF:(h + 1) * HALF],
                    start=True, stop=True,
                )
                # DVE may read only one PSUM operand per op: stage block 0
                # to SBUF on the Scalar engine first.
                s0 = wpool.tile([RANK, HALF], F32, name=f"s0_{h}", tag="s0")
                nc.scalar.copy(out=s0[:], in_=pv[0:32, :])
                v01 = wpool.tile([RANK, HALF], F32, name=f"v01_{h}", tag="v01")
                nc.vector.tensor_tensor(
                    out=v01[:], in0=s0[:], in1=pv[32:64, :], op=ALU.mult
                )
                vth = cpool.tile([RANK, HALF], MM_DT, name=f"vt_{h}")
                nc.vector.tensor_tensor(
                    out=vth[:], in0=v01[:], in1=pv[64:96, :], op=ALU.mult
                )

                # two output chunks batched per [128, 1024] psum pair
                for g in range(NC2):
                    po2 = oppool.tile([128, 2 * EMB], F32, name=f"po_{h}{g}",
                                      tag="po")
                    for j in range(2):
                        nc.tensor.matmul(
                            po2[:, j * EMB:(j + 1) * EMB],
                            lhsT=vth[:, (2 * g + j) * 128:(2 * g + j + 1) * 128],
                            rhs=wt[:],
                            start=True, stop=True,
                        )
                    osb = opool.tile([128, 2 * EMB], F32, name=f"osb_{h}{g}",
                                     tag="osb")
                    if g == 0:
                        nc.scalar.copy(out=osb[:], in_=po2[:])
                    else:
                        nc.vector.tensor_copy(out=osb[:], in_=po2[:])
                    row0 = h * HALF + g * 256
                    nc.sync.dma_start(
                        out=out[row0:row0 + 256, :].rearrange(
                            "(j p) e -> p j e", p=128
                        ),
                        in_=osb[:].rearrange("p (j e) -> p j e", j=2),
                    )

    nc.compile()
    return nc


_CACHE: dict = {}


def _get_nc():
    if "nc" not in _CACHE:
        _CACHE["nc"] = build()
    return _CACHE["nc"]


def run(inputs, **spmd_kwargs):
    nc = _get_nc()
    x = np.ascontiguousarray(inputs["x"].reshape(-1), dtype=np.int32)
    us = [
        np.ascontiguousarray(inputs[f"U{j}"], dtype=np.float32) for j in range(6)
    ]
    aux = _aux_table(us)
    in_maps = []
    for i in range(N_CORES):
        in_maps.append({"x": x[i * PER_CORE:(i + 1) * PER_CORE], "aux": aux})
    res = run_bass_kernel_spmd(
        nc, in_maps, core_ids=list(range(N_CORES)), **spmd_kwargs
    )
    shards = [np.asarray(res.results[i]["out"]) for i in range(N_CORES)]
    full = np.concatenate(shards, axis=0).reshape(4, 2048, EMB)
    return full.astype(np.float32, copy=False), res


def kernel(**inputs) -> np.ndarray:
    return run(inputs)[0]



# revision 17
# speedup vs baseline: 1.2904x; 1.2904x over previous
"""CP-factorized embedding lookup on 8 TRN2 NeuronCores.

Reference computes full[a,b,c,d,e,f] = sum_r U0[a,r]*...*U5[f,r], reshapes to a
(50000, 512) table, and gathers rows by x. We never materialize the table:

  out[n, e] = sum_r (U0[a_n,r]*U1[b_n,r]*U2[c_n,r]) * W[e, r],  W = KR(U3,U4,U5)

with v = 1000a + 25b + c.  Per core: 1024 indices (data-parallel over 8192).

Pipeline (all arithmetic on device; host only packs/replicates constants):
  1. x arrives as a [16, 256] tile (4 column-chunks x 4 digit-slots, host-tiled).
     Digit ops on [4, 256] tiles: a = rint((v-499.5)/1000), q25 = rint((v-12)/25)
     (f32->i16 store rounds to nearest; bias puts the value mid-interval so the
     result is exact), b0/b1 = low/high bytes of v via a uint8 bitcast view.
     All digit values < 2048 so they are exact in f32r (tf32).
  2. One small PE matmul per 256-column chunk combines + broadcasts digits into
     psumD[116, n]: rows 0:50 get a, 50:90 get q25-40a = b, 90:115 get
     b0+256*b1-25*q25 = c, row 115 gets a+b+c (zero iff v==0: padding mask).
  3. onehot[116, n] = is_equal(psumD, targ_p) * sgn_p  -- one DVE op per half.
     sgn[115] = -1; ublk row 115 carries U0[0,:], so the gather computes
     P0' = U0[a] - (v==0)*U0[0], zeroing padded rows end-to-end.
  4. gather matmul: psum2[96, n] = blockdiag[U0;U1;U2]^T @ onehot.
  5. V chain in bf16: pb = copy(psum2) (column-interleaved so output rows land
     n = 4m+k), v01 = P0'*P1, vth[32k:32k+32] = chunk k of v01*P2.
  6. W[32, 512] = Khatri-Rao of host-transposed U3,U4,U5, replicated to 4
     row-groups; 4 output matmuls per half run concurrently via row-group
     packing (K=32 each).  psum -> bf16 SBUF -> one 512KB DMA per half with
     4KB-contiguous descriptors; host upcasts bf16 -> fp32.
"""

import numpy as np

import concourse.bass as bass
import concourse.mybir as mybir
import concourse.tile as tile
from concourse import bacc
from concourse.bass_utils import run_bass_kernel_spmd

F32 = mybir.dt.float32
BF16 = mybir.dt.bfloat16
FP16 = mybir.dt.float16
I32 = mybir.dt.int32
I16 = mybir.dt.int16
U8 = mybir.dt.uint8
ALU = mybir.AluOpType

N_CORES = 8
PER_CORE = 1024           # indices per core (8192 / 8)
HALF = 512
EMB = 512
RANK = 32
KV = 116                  # 50 + 40 + 25 factor rows + 1 padding-mask row
NCH = 4                   # x column chunks
CHW = 256                 # chunk width

R1000 = float(np.float32(1.0 / 1000.0))
R25 = float(np.float32(1.0 / 25.0))

# aux layout: [128, AUXW] fp32 (16-bit blocks bit-packed, read via bitcast)
TARG = 0                  # col 0: one-hot comparison target per partition
SGN = 1                   # col 1: +1, except -1 on the mask row
U345T = 2                 # [32, 24] fp32: U3^T | U4^T | U5^T
UBLK = 26                 # [116, 48] fp32 = [116, 96] bf16 blockdiag(U0,U1,U2)
SEL = 74                  # 4 x ([128, 58] fp32 = [128, 116] fp16) digit combos
AUXW = SEL + 4 * (KV // 2)  # 306


def _pack16(cols16: np.ndarray) -> np.ndarray:
    """Pack an even-width 16-bit array into fp32 bit-pair columns."""
    u16 = np.ascontiguousarray(cols16).view(np.uint16)
    lo = u16[:, 0::2].astype(np.uint32)
    hi = u16[:, 1::2].astype(np.uint32)
    return ((hi << 16) | lo).view(np.float32)


def _aux_table(us: list[np.ndarray]) -> np.ndarray:
    aux = np.zeros((128, AUXW), np.float32)
    p = np.arange(KV, dtype=np.float32)
    targ = np.where(p < 50, p, np.where(p < 90, p - 50, p - 90))
    targ[115] = 0.0
    aux[0:KV, TARG] = targ
    aux[0:KV, SGN] = 1.0
    aux[115, SGN] = -1.0
    aux[0:RANK, U345T:U345T + 8] = us[3].T
    aux[0:RANK, U345T + 8:U345T + 16] = us[4].T
    aux[0:RANK, U345T + 16:U345T + 24] = us[5].T
    ublk = np.zeros((KV, 96), np.float32)
    ublk[0:50, 0:32] = us[0]
    ublk[50:90, 32:64] = us[1]
    ublk[90:115, 64:96] = us[2]
    ublk[115, 0:32] = us[0][0]   # mask row: P0' = U0[a] - m*U0[0]
    # bf16 = upper 16 bits of fp32 with round-to-nearest-even
    ub = ublk.view(np.uint32)
    ubf = ((ub + 0x7FFF + ((ub >> 16) & 1)) >> 16).astype(np.uint16)
    aux[0:KV, UBLK:UBLK + 48] = _pack16(ubf)
    # digit combination coefficients: digits (a, q25, b0, b1)
    coeff = np.zeros((4, KV), np.float32)
    coeff[0, 0:50] = 1.0                  # a
    coeff[0, 50:90] = -40.0               # b = q25 - 40a
    coeff[0, 115] = -39.0                 # a+b+c = -39a - 24q25 + b0 + 256b1
    coeff[1, 50:90] = 1.0
    coeff[1, 90:115] = -25.0              # c = b0 + 256b1 - 25q25
    coeff[1, 115] = -24.0
    coeff[2, 90:115] = 1.0
    coeff[2, 115] = 1.0
    coeff[3, 90:115] = 256.0
    coeff[3, 115] = 256.0
    # sel_c: [128, 116] fp16; row 32d+c' is coeff[d] if c'==c else 0
    for c in range(NCH):
        sel = np.zeros((128, KV), np.float16)
        for d in range(4):
            sel[32 * d + c] = coeff[d].astype(np.float16)
        base = SEL + (KV // 2) * c
        aux[:, base:base + KV // 2] = _pack16(sel)
    return aux


def build():
    nc = bacc.Bacc("TRN2", target_bir_lowering=False, debug=False)

    x = nc.dram_tensor("x", [128, CHW], I32, kind="ExternalInput")
    aux_d = nc.dram_tensor("aux", [128, AUXW], F32, kind="ExternalInput")
    out = nc.dram_tensor("out", [PER_CORE, EMB], BF16, kind="ExternalOutput")

    with tile.TileContext(nc) as tc:
        with (
            tc.tile_pool(name="const", bufs=1) as cpool,
            tc.tile_pool(name="work", bufs=2) as wpool,
            tc.tile_pool(name="pD", bufs=1, space="PSUM") as pD,
            tc.tile_pool(name="pG", bufs=2, space="PSUM") as pG,
            tc.tile_pool(name="pO", bufs=1, space="PSUM") as pO,
            tc.tile_pool(name="osb", bufs=2) as opool,
        ):
            aux = cpool.tile([128, AUXW], F32)
            nc.sync.dma_start(out=aux[:], in_=aux_d[:])
            xrep = cpool.tile([128, CHW], I32)
            nc.scalar.dma_start(out=xrep[:], in_=x[:])

            # ---- digits: a, q25 (rint via i16 store), b0, b1 (byte views);
            # one digit per 32-row block (engine operand bases are 32-aligned)
            aq = cpool.tile([64, CHW], I16)
            nc.vector.tensor_scalar(
                out=aq[0:32], in0=xrep[0:32], scalar1=-499.5, scalar2=R1000,
                op0=ALU.add, op1=ALU.mult,
            )
            nc.vector.tensor_scalar(
                out=aq[32:64], in0=xrep[32:64], scalar1=-12.0, scalar2=R25,
                op0=ALU.add, op1=ALU.mult,
            )
            dig = cpool.tile([128, CHW], FP16)
            nc.gpsimd.tensor_copy(out=dig[0:64], in_=aq[:])
            x8 = xrep[:].bitcast(U8).rearrange("p (n k) -> p k n", k=4)
            nc.scalar.copy(out=dig[64:96], in_=x8[64:96, 0, :])    # b0 (LSB)
            nc.scalar.copy(out=dig[96:128], in_=x8[96:128, 1, :])  # b1

            # ---- W[r, e] = U3[d,r]*U4[e2,r]*U5[f,r], replicated to 4 groups
            u3t = aux[0:RANK, U345T:U345T + 8]
            u4t = aux[0:RANK, U345T + 8:U345T + 16]
            u5t = aux[0:RANK, U345T + 16:U345T + 24]
            t45 = cpool.tile([RANK, 64], F32)
            nc.gpsimd.tensor_tensor(
                out=t45[:].rearrange("r (e f) -> r e f", e=8),
                in0=u4t.unsqueeze(2).broadcast_to([RANK, 8, 8]),
                in1=u5t.unsqueeze(1).broadcast_to([RANK, 8, 8]),
                op=ALU.mult,
            )
            wt4 = cpool.tile([128, EMB], BF16)
            nc.vector.tensor_tensor(
                out=wt4[0:RANK, :].rearrange("r (d ef) -> r d ef", d=8),
                in0=u3t.unsqueeze(2).broadcast_to([RANK, 8, 64]),
                in1=t45[:].unsqueeze(1).broadcast_to([RANK, 8, 64]),
                op=ALU.mult,
            )
            nc.scalar.copy(out=wt4[32:64, :], in_=wt4[0:32, :])
            nc.vector.tensor_copy(out=wt4[64:96, :], in_=wt4[0:32, :])
            nc.scalar.copy(out=wt4[96:128, :], in_=wt4[0:32, :])

            # ---- combine/broadcast digits: psumD[p, n] per 256-col chunk
            psumD = pD.tile([KV, PER_CORE], F32)
            SELW = KV // 2
            for c in range(NCH):
                sel = aux[:, SEL + SELW * c:SEL + SELW * (c + 1)].bitcast(FP16)
                nc.tensor.matmul(
                    psumD[:, c * CHW:(c + 1) * CHW],
                    lhsT=sel, rhs=dig[:], start=True, stop=True,
                )

            ublk = aux[0:KV, UBLK:UBLK + 48].bitcast(BF16)
            targ = aux[0:KV, TARG:TARG + 1]
            sgn = aux[0:KV, SGN:SGN + 1]

            for h in range(2):
                # one-hot (mask row gets -(v==0))
                oh = wpool.tile([KV, HALF], BF16, name=f"oh_{h}", tag="oh")
                nc.vector.tensor_scalar(
                    out=oh[:], in0=psumD[:, h * HALF:(h + 1) * HALF],
                    scalar1=targ, scalar2=sgn,
                    op0=ALU.is_equal, op1=ALU.mult,
                )
                # gather the three factor rows
                ps2 = pG.tile([96, HALF], F32, name=f"ps2_{h}", tag="g")
                nc.tensor.matmul(
                    ps2[:], lhsT=ublk, rhs=oh[:], start=True, stop=True,
                )
                # V chain in bf16; columns interleaved so chunk k holds n=4m+k
                pb = wpool.tile([96, HALF], BF16, name=f"pb_{h}", tag="pb")
                nc.scalar.copy(
                    out=pb[:].rearrange("p (k m) -> p k m", k=4),
                    in_=ps2[:].rearrange("p (m k) -> p k m", k=4),
                )
                # re-base P1/P2 to partition 0 (SB+SB ops need equal bases)
                pb1 = wpool.tile([RANK, HALF], BF16, name=f"pb1_{h}", tag="pb1")
                nc.vector.tensor_copy(out=pb1[:], in_=pb[32:64, :])
                pb2 = wpool.tile([RANK, HALF], BF16, name=f"pb2_{h}", tag="pb2")
                nc.vector.tensor_copy(out=pb2[:], in_=pb[64:96, :])
                v01 = wpool.tile([RANK, HALF], BF16, name=f"v01_{h}", tag="v01")
                nc.vector.tensor_tensor(
                    out=v01[:], in0=pb[0:32, :], in1=pb1[:], op=ALU.mult
                )
                vth = wpool.tile([128, 128], BF16, name=f"vth_{h}", tag="vth")
                for k in range(4):
                    nc.vector.tensor_tensor(
                        out=vth[32 * k:32 * (k + 1), :],
                        in0=v01[:, 128 * k:128 * (k + 1)],
                        in1=pb2[:, 128 * k:128 * (k + 1)],
                        op=ALU.mult,
                    )
                # 4 output matmuls, row-group packed (K=32 each)
                poA = pO.tile([128, 2 * EMB], F32, name=f"poA_{h}", tag="poA")
                poB = pO.tile([128, 2 * EMB], F32, name=f"poB_{h}", tag="poB")
                for k in range(4):
                    po = poA if k < 2 else poB
                    nc.tensor.matmul(
                        po[:, (k % 2) * EMB:(k % 2 + 1) * EMB],
                        lhsT=vth[32 * k:32 * (k + 1), :],
                        rhs=wt4[32 * k:32 * (k + 1), :],
                        start=True, stop=True,
                        tile_position=(32 * k, 0),
                    )
                osb = opool.tile([128, 4 * EMB], BF16, name=f"osb_{h}",
                                 tag="osb")
                nc.scalar.copy(out=osb[:, 0:2 * EMB], in_=poA[:])
                nc.vector.tensor_copy(out=osb[:, 2 * EMB:4 * EMB], in_=poB[:])
                eng = nc.sync if h == 0 else nc.scalar
                eng.dma_start(
                    out=out[h * 4 * 128:(h + 1) * 4 * 128, :].rearrange(
                        "(p j) e -> p (j e)", j=4
                    ),
                    in_=osb[:],
                )

    nc.compile()
    return nc


_CACHE: dict = {}


def _get_nc():
    if "nc" not in _CACHE:
        _CACHE["nc"] = build()
    return _CACHE["nc"]


def run(inputs, **spmd_kwargs):
    nc = _get_nc()
    x = np.ascontiguousarray(inputs["x"].reshape(-1), dtype=np.int32)
    us = [
        np.ascontiguousarray(inputs[f"U{j}"], dtype=np.float32) for j in range(6)
    ]
    aux = _aux_table(us)
    in_maps = []
    for i in range(N_CORES):
        xc = x[i * PER_CORE:(i + 1) * PER_CORE].reshape(NCH, CHW)
        x128 = np.zeros((128, CHW), np.int32)
        for g in range(4):
            x128[32 * g:32 * g + NCH] = xc
        in_maps.append({"x": x128, "aux": aux})
    res = run_bass_kernel_spmd(
        nc, in_maps, core_ids=list(range(N_CORES)), **spmd_kwargs
    )
    shards = [np.asarray(res.results[i]["out"]) for i in range(N_CORES)]
    full = np.concatenate(shards, axis=0).reshape(4, 2048, EMB)
    return full.astype(np.float32), res


def kernel(**inputs) -> np.ndarray:
    return run(inputs)[0]


# revision 23
# speedup vs baseline: 1.3356x; 1.0350x over previous
"""CP-factorized embedding lookup on 8 TRN2 NeuronCores.

Reference computes full[a,b,c,d,e,f] = sum_r U0[a,r]*...*U5[f,r], reshapes to a
(50000, 512) table, and gathers rows by x. We never materialize the table:

  out[n, e] = sum_r (U0[a_n,r]*U1[b_n,r]*U2[c_n,r]) * W[e, r],  W = KR(U3,U4,U5)

with v = 1000a + 25b + c.  Per core: 1024 indices (data-parallel over 8192).

Pipeline (all arithmetic on device; host only packs/replicates constants):
  1. x arrives as a [128, 256] tile: 4 column-chunks replicated into four
     32-row digit blocks (rows beyond the 4 chunks are zero).  Digit ops:
     a = rint((v-499.5)/1000) and q25 = rint((v-12)/25) via i16 stores (the
     f32->i16 cast rounds to nearest; the bias puts the value mid-interval so
     the result is exact -- verified exhaustively), b0/b1 = low/high bytes of
     v via a uint8 bitcast view.  All digits < 2048 so they are exact fp16.
  2. One small PE matmul per 256-column chunk combines + broadcasts digits
     into psumD[116, n]: rows 0:50 get a, 50:90 get q25-40a = b, 90:115 get
     b0+256*b1-25*q25 = c, row 115 gets a+b+c (zero iff v==0: padding mask).
  3. onehot[116, n] = is_equal(psumD, targ_p) * sgn_p  -- one DVE op per half.
     sgn[115] = -1; ublk row 115 carries U0[0,:], so the gather computes
     P0' = U0[a] - (v==0)*U0[0], zeroing padded rows end-to-end.
  4. gather matmul: psum2[96, n] = blockdiag[U0;U1;U2]^T @ onehot (bf16).
  5. V chain: pb = bf16 copy of P0' (column-interleaved so output rows land
     n = 4m+k), v01 = pb * P1(psum), vth[32k:32k+32] = v01 * P2(psum) chunks.
  6. W[32, 512] = Khatri-Rao of host-transposed U3,U4,U5 (gpsimd), replicated
     to rows 32:128 by one PE matmul against [I;I;I] + one scalar copy; the
     4 output matmuls per half then run concurrently via row-group packing
     (K=32 each).  psum -> bf16 SBUF -> two 256KB DMAs per half with
     2KB-contiguous descriptors; host upcasts bf16 -> fp32.
"""

import numpy as np

import concourse.bass as bass
import concourse.mybir as mybir
import concourse.tile as tile
from concourse import bacc
from concourse.bass_utils import run_bass_kernel_spmd

F32 = mybir.dt.float32
BF16 = mybir.dt.bfloat16
FP16 = mybir.dt.float16
I32 = mybir.dt.int32
I16 = mybir.dt.int16
U8 = mybir.dt.uint8
ALU = mybir.AluOpType

N_CORES = 8
PER_CORE = 1024           # indices per core (8192 / 8)
HALF = 512
EMB = 512
RANK = 32
KV = 116                  # 50 + 40 + 25 factor rows + 1 padding-mask row
NCH = 4                   # x column chunks
CHW = 256                 # chunk width

R1000 = float(np.float32(1.0 / 1000.0))
R25 = float(np.float32(1.0 / 25.0))

# aux layout: [128, AUXW] fp32 (16-bit blocks bit-packed, read via bitcast)
TARG = 0                  # col 0: one-hot comparison target per partition
SGN = 1                   # col 1: +1, except -1 on the mask row
U345T = 2                 # [32, 24] fp32: U3^T | U4^T | U5^T
UBLK = 26                 # [116, 48] fp32 = [116, 96] bf16 blockdiag(U0,U1,U2)
I4B = 74                  # [32, 64] fp32 = [32, 128] bf16 [I32 I32 I32 I32]
SEL = 138                 # 4 x ([128, 58] fp32 = [128, 116] fp16) digit combos
AUXW = SEL + 4 * (KV // 2)  # 370


def _pack16(cols16: np.ndarray) -> np.ndarray:
    """Pack an even-width 16-bit array into fp32 bit-pair columns."""
    u16 = np.ascontiguousarray(cols16).view(np.uint16)
    lo = u16[:, 0::2].astype(np.uint32)
    hi = u16[:, 1::2].astype(np.uint32)
    return ((hi << 16) | lo).view(np.float32)


def _bf16_bits(a: np.ndarray) -> np.ndarray:
    """fp32 -> bf16 bits (round to nearest even), as uint16."""
    u = np.ascontiguousarray(a, np.float32).view(np.uint32)
    return ((u + 0x7FFF + ((u >> 16) & 1)) >> 16).astype(np.uint16)


def _aux_table(us: list[np.ndarray]) -> np.ndarray:
    aux = np.zeros((128, AUXW), np.float32)
    p = np.arange(KV, dtype=np.float32)
    targ = np.where(p < 50, p, np.where(p < 90, p - 50, p - 90))
    targ[115] = 0.0
    aux[0:KV, TARG] = targ
    aux[0:KV, SGN] = 1.0
    aux[115, SGN] = -1.0
    aux[0:RANK, U345T:U345T + 8] = us[3].T
    aux[0:RANK, U345T + 8:U345T + 16] = us[4].T
    aux[0:RANK, U345T + 16:U345T + 24] = us[5].T
    ublk = np.zeros((KV, 96), np.float32)
    ublk[0:50, 0:32] = us[0]
    ublk[50:90, 32:64] = us[1]
    ublk[90:115, 64:96] = us[2]
    ublk[115, 0:32] = us[0][0]   # mask row: P0' = U0[a] - m*U0[0]
    aux[0:KV, UBLK:UBLK + 48] = _pack16(_bf16_bits(ublk))
    i4 = np.zeros((RANK, 128), np.float32)
    for j in range(4):
        i4[:, 32 * j:32 * (j + 1)] = np.eye(RANK, dtype=np.float32)
    aux[0:RANK, I4B:I4B + 64] = _pack16(_bf16_bits(i4))
    # digit combination coefficients: digits (a, q25, b0, b1)
    coeff = np.zeros((4, KV), np.float32)
    coeff[0, 0:50] = 1.0                  # a
    coeff[0, 50:90] = -40.0               # b = q25 - 40a
    coeff[0, 115] = -39.0                 # a+b+c = -39a - 24q25 + b0 + 256b1
    coeff[1, 50:90] = 1.0
    coeff[1, 90:115] = -25.0              # c = b0 + 256b1 - 25q25
    coeff[1, 115] = -24.0
    coeff[2, 90:115] = 1.0
    coeff[2, 115] = 1.0
    coeff[3, 90:115] = 256.0
    coeff[3, 115] = 256.0
    # sel_c: [128, 116] fp16; row 32d+c' is coeff[d] if c'==c else 0
    for c in range(NCH):
        sel = np.zeros((128, KV), np.float16)
        for d in range(4):
            sel[32 * d + c] = coeff[d].astype(np.float16)
        base = SEL + (KV // 2) * c
        aux[:, base:base + KV // 2] = _pack16(sel)
    return aux


def build():
    nc = bacc.Bacc("TRN2", target_bir_lowering=False, debug=False)

    x = nc.dram_tensor("x", [128, CHW], I32, kind="ExternalInput")
    aux_d = nc.dram_tensor("aux", [128, AUXW], F32, kind="ExternalInput")
    out = nc.dram_tensor("out", [PER_CORE, EMB], BF16, kind="ExternalOutput")

    with tile.TileContext(nc) as tc:
        with (
            tc.tile_pool(name="const", bufs=1) as cpool,
            tc.tile_pool(name="work", bufs=2) as wpool,
            tc.tile_pool(name="pD", bufs=1, space="PSUM") as pD,
            tc.tile_pool(name="pG", bufs=2, space="PSUM") as pG,
            tc.tile_pool(name="pO", bufs=1, space="PSUM") as pO,
            tc.tile_pool(name="osb", bufs=2) as opool,
        ):
            aux = cpool.tile([128, AUXW], F32)
            nc.sync.dma_start(out=aux[:], in_=aux_d[:])
            xrep = cpool.tile([128, CHW], I32)
            nc.scalar.dma_start(out=xrep[:], in_=x[:])

            # ---- digits: a, q25 (rint via i16 store), b0, b1 (byte views);
            # one digit per 32-row block (engine operand bases are 32-aligned)
            aq = cpool.tile([64, CHW], I16)
            nc.vector.tensor_scalar(
                out=aq[0:32], in0=xrep[0:32], scalar1=-499.5, scalar2=R1000,
                op0=ALU.add, op1=ALU.mult,
            )
            nc.vector.tensor_scalar(
                out=aq[32:64], in0=xrep[32:64], scalar1=-12.0, scalar2=R25,
                op0=ALU.add, op1=ALU.mult,
            )
            dig = cpool.tile([128, CHW], FP16)
            nc.vector.tensor_copy(out=dig[0:64], in_=aq[:])
            x8 = xrep[:].bitcast(U8).rearrange("p (n k) -> p k n", k=4)
            nc.scalar.copy(out=dig[64:96], in_=x8[64:96, 0, :])    # b0 (LSB)
            nc.scalar.copy(out=dig[96:128], in_=x8[96:128, 1, :])  # b1

            # ---- W[r, e] = U3[d,r]*U4[e2,r]*U5[f,r] on gpsimd (idle early)
            u3t = aux[0:RANK, U345T:U345T + 8]
            u4t = aux[0:RANK, U345T + 8:U345T + 16]
            u5t = aux[0:RANK, U345T + 16:U345T + 24]
            t45 = cpool.tile([RANK, 64], F32)
            nc.gpsimd.tensor_tensor(
                out=t45[:].rearrange("r (e f) -> r e f", e=8),
                in0=u4t.unsqueeze(2).broadcast_to([RANK, 8, 8]),
                in1=u5t.unsqueeze(1).broadcast_to([RANK, 8, 8]),
                op=ALU.mult,
            )
            wt1 = cpool.tile([RANK, EMB], BF16)
            nc.gpsimd.tensor_tensor(
                out=wt1[:].rearrange("r (d ef) -> r d ef", d=8),
                in0=u3t.unsqueeze(2).broadcast_to([RANK, 8, 64]),
                in1=t45[:].unsqueeze(1).broadcast_to([RANK, 8, 64]),
                op=ALU.mult,
            )
            wt4 = cpool.tile([128, EMB], BF16)

            # ---- combine/broadcast digits: psumD[p, n] per 256-col chunk
            psumD = pD.tile([KV, PER_CORE], F32, name="psumD", tag="d")
            SELW = KV // 2
            for c in range(NCH):
                sel = aux[:, SEL + SELW * c:SEL + SELW * (c + 1)].bitcast(FP16)
                nc.tensor.matmul(
                    psumD[:, c * CHW:(c + 1) * CHW],
                    lhsT=sel, rhs=dig[:], start=True, stop=True,
                )

            ublk = aux[0:KV, UBLK:UBLK + 48].bitcast(BF16)
            i4b = aux[0:RANK, I4B:I4B + 64].bitcast(BF16)
            targ = aux[0:KV, TARG:TARG + 1]
            sgn = aux[0:KV, SGN:SGN + 1]

            ohs, ps2s = [], []
            for h in range(2):
                # one-hot (mask row gets -(v==0))
                oh = wpool.tile([KV, HALF], BF16, name=f"oh_{h}", tag="oh")
                nc.vector.tensor_scalar(
                    out=oh[:], in0=psumD[:, h * HALF:(h + 1) * HALF],
                    scalar1=targ, scalar2=sgn,
                    op0=ALU.is_equal, op1=ALU.mult,
                )
                ohs.append(oh)
                # gather the three factor rows
                ps2 = pG.tile([96, HALF], F32, name=f"ps2_{h}", tag="g")
                nc.tensor.matmul(
                    ps2[:], lhsT=ublk, rhs=oh[:], start=True, stop=True,
                )
                ps2s.append(ps2)

            # replicate W to all 4 row blocks via PE (reuses psumD banks)
            wrep = pD.tile([128, EMB], F32, name="wrep", tag="d")
            nc.tensor.matmul(
                wrep[:], lhsT=i4b, rhs=wt1[:], start=True, stop=True,
            )
            nc.scalar.copy(out=wt4[:], in_=wrep[:])

            for h in range(2):
                ps2 = ps2s[h]
                # V chain; columns interleaved so chunk k holds n = 4m+k
                pb = wpool.tile([RANK, HALF], BF16, name=f"pb_{h}", tag="pb")
                nc.scalar.copy(
                    out=pb[:].rearrange("p (k m) -> p k m", k=4),
                    in_=ps2[0:32, :].rearrange("p (m k) -> p k m", k=4),
                )
                v01 = wpool.tile([RANK, HALF], BF16, name=f"v01_{h}", tag="v01")
                nc.vector.tensor_tensor(
                    out=v01[:].rearrange("p (k m) -> p k m", k=4),
                    in0=pb[:].rearrange("p (k m) -> p k m", k=4),
                    in1=ps2[32:64, :].rearrange("p (m k) -> p k m", k=4),
                    op=ALU.mult,
                )
                vth = wpool.tile([128, 128], BF16, name=f"vth_{h}", tag="vth")
                p2v = ps2[64:96, :].rearrange("p (m k) -> p k m", k=4)
                for k in range(4):
                    nc.vector.tensor_tensor(
                        out=vth[32 * k:32 * (k + 1), :],
                        in0=v01[:, 128 * k:128 * (k + 1)],
                        in1=p2v[:, k, :],
                        op=ALU.mult,
                    )
                # 4 output matmuls, row-group packed (K=32 each)
                poA = pO.tile([128, 2 * EMB], F32, name=f"poA_{h}", tag="poA")
                if h == 0:
                    poB = pO.tile([128, 2 * EMB], F32, name="poB_0", tag="poB")
                else:
                    poB = pD.tile([128, 2 * EMB], F32, name="poB_1", tag="d")
                for k in range(4):
                    po = poA if k < 2 else poB
                    nc.tensor.matmul(
                        po[:, (k % 2) * EMB:(k % 2 + 1) * EMB],
                        lhsT=vth[32 * k:32 * (k + 1), :],
                        rhs=wt4[32 * k:32 * (k + 1), :],
                        start=True, stop=True,
                        tile_position=(32 * k, 0),
                    )
                osb = opool.tile([128, 4 * EMB], BF16, name=f"osb_{h}",
                                 tag="osb")
                nc.scalar.copy(out=osb[:, 0:2 * EMB], in_=poA[:])
                nc.vector.tensor_copy(out=osb[:, 2 * EMB:4 * EMB], in_=poB[:])
                dst = out[h * HALF:(h + 1) * HALF, :].rearrange(
                    "(p j) e -> p (j e)", j=4
                )
                engA = nc.sync if h == 0 else nc.scalar
                engA.dma_start(out=dst[:, 0:2 * EMB], in_=osb[:, 0:2 * EMB])
                nc.sync.dma_start(
                    out=dst[:, 2 * EMB:4 * EMB], in_=osb[:, 2 * EMB:4 * EMB]
                )

    nc.compile()
    return nc


_CACHE: dict = {}


def _get_nc():
    if "nc" not in _CACHE:
        _CACHE["nc"] = build()
    return _CACHE["nc"]


def run(inputs, **spmd_kwargs):
    nc = _get_nc()
    x = np.ascontiguousarray(inputs["x"].reshape(-1), dtype=np.int32)
    us = [
        np.ascontiguousarray(inputs[f"U{j}"], dtype=np.float32) for j in range(6)
    ]
    aux = _aux_table(us)
    in_maps = []
    for i in range(N_CORES):
        xc = x[i * PER_CORE:(i + 1) * PER_CORE].reshape(NCH, CHW)
        x128 = np.zeros((128, CHW), np.int32)
        for g in range(4):
            x128[32 * g:32 * g + NCH] = xc
        in_maps.append({"x": x128, "aux": aux})
    res = run_bass_kernel_spmd(
        nc, in_maps, core_ids=list(range(N_CORES)), **spmd_kwargs
    )
    shards = [np.asarray(res.results[i]["out"]) for i in range(N_CORES)]
    full = np.concatenate(shards, axis=0).reshape(4, 2048, EMB)
    return full.astype(np.float32), res


def kernel(**inputs) -> np.ndarray:
    return run(inputs)[0]
